# revision 17
# baseline (speedup 1.0000x reference)
"""Trainium2 Bass kernel for nn_Attention_8143257993917.

Multi-head attention (packed QKV + RoPE + additive bias + softmax + head_mask
+ o_proj), B=4, S=2048, D=1024, H=16 heads, fp32 I/O.

Sharding: 8 cores = 4 batches x 2 head-groups (tensor-parallel over heads).
Core c handles batch b = c // 2 and heads g*8..g*8+8 with g = c % 2.
Each core computes a partial output (its heads' contribution through o_proj);
the host sums the two partials per batch and adds o_b.

v2 design (per core), all feature-major layouts, fp16 matmul datapath with
fp32 PSUM accumulation:
  Phase A (projections + rope):
    Q_T/K_T [f, t]: psum = w_tile.T @ hT, evacuated on ScalarE with the bias
    add folded in (activation Copy + per-partition bias), then
    q' = qpb*cos + rot(qpb)*sin where rot(qpb) comes from a single [128,128]
    signed-permutation matmul (rotate_half folds into a constant matrix and
    the bias is already inside qpb). N=1024 moving operands.
    V [t, f] via stationary-h matmuls (so V chunks are directly the PV
    stationary); ones-row matmul adds the (head_mask-folded) V bias; a
    ones-column appended to V makes PV also produce softmax denominators.
  Phase B (attention), loop qh(2) outer x hp(4), per-kc software pipeline:
    scores sc_hi[k,q] = K_chunk.T @ Q (row-tiled 64x128: h-even on array
    rows 0-63, h-odd on 64-127), exp on ScalarE (PSUM->SBUF, constant -12
    shift), u2 = exp(s)*exp(bias) on DVE (exp(bias) precomputed on host,
    fp16, loaded once per qh half), PV lags one kc. Split per-head exp
    instructions let head h0's next-kc scores matmul run while h1's exp
    occupies ScalarE, keeping ScalarE (the throughput floor: ~33.6M exp
    elems/core, ScalarE-only) saturated. PSUM: 2 scores + 2 ctx tiles = 8
    banks exactly.
    Denominator reciprocal via DRAM round-trip reshape + partition
    broadcast on the gpsimd DMA queues (as baseline).
  Phase C: o_proj out_T[o,t] = sum_fc ow.T @ ctxT, fp16 output (host sums
    partials in fp32).
"""

import sys

sys.path.insert(0, "/opt/trn_rl_repo")

import numpy as np

_CACHE = {}

H = 16
HPC = 8  # heads per core
G = 2  # head groups


def build_nc(S=2048, D=1024):
    import concourse.bass as bass
    from concourse import bacc
    import concourse.mybir as mybir
    import concourse.tile as tile

    F32 = mybir.dt.float32
    F16 = mybir.dt.float16
    AF = mybir.ActivationFunctionType

    P = 128
    DC = D // P           # contraction chunks for projections (8)
    KC = S // P           # k chunks (16)
    NQ = 512              # matmul moving free-dim max / q quarter
    NQQ = S // NQ         # q quarters (4)
    FPC = HPC * 64        # features per core (512)
    FT = FPC // P         # f-tiles per tensor (4)

    nc = bacc.Bacc("TRN2", target_bir_lowering=False, debug=False, num_devices=8)

    hT = nc.dram_tensor("hT", [D, S], F16, kind="ExternalInput")
    w4 = nc.dram_tensor("w4", [D, 2 * FPC], F16, kind="ExternalInput")
    b4 = nc.dram_tensor("b4", [2 * FPC], F32, kind="ExternalInput")
    wvT = nc.dram_tensor("wvT", [D, FPC], F16, kind="ExternalInput")
    cosr = nc.dram_tensor("cosr", [P, S], F16, kind="ExternalInput")
    sinr = nc.dram_tensor("sinr", [P, S], F16, kind="ExternalInput")
    permM = nc.dram_tensor("permM", [P, P], F16, kind="ExternalInput")
    expbT = nc.dram_tensor("expbT", [S, S], F16, kind="ExternalInput")
    owT = nc.dram_tensor("owT", [FPC, D], F16, kind="ExternalInput")
    outT = nc.dram_tensor("outT", [D, S], F16, kind="ExternalOutput")

    hT_r = hT.ap().rearrange("(o p) t -> p o t", p=P)
    w4_r = w4.ap().rearrange("(o p) f -> p o f", p=P)
    wv_r = wvT.ap().rearrange("(o p) f -> p o f", p=P)
    ow_r = owT.ap().rearrange("(o p) f -> p o f", p=P)
    b4_r = b4.ap().rearrange("(o p) -> p o", p=P)
    eb_r = expbT.ap().rearrange("(kc p) q -> p kc q", p=P)

    with tile.TileContext(nc) as tc:
        with (
            tc.tile_pool(name="cst", bufs=1) as cst,
            tc.tile_pool(name="pAB", bufs=1) as pAB,
            tc.tile_pool(name="pA", bufs=1) as pA,
            tc.tile_pool(name="dram", bufs=4, space="DRAM") as dpool,
        ):
            b4_sb = cst.tile([P, 2 * FPC // P], F32)
            nc.sync.dma_start(b4_sb[:], b4_r)
            eshift = cst.tile([P, 1], F32)
            nc.vector.memset(eshift[:], -12.0)
            permM_sb = cst.tile([P, P], F16)
            nc.sync.dma_start(permM_sb[:], permM.ap())

            # persistent phase-A products
            qk_sb = pAB.tile([P, 2 * FT, S], F16)      # Q ft 0..3, K ft 4..7
            v_sb = pAB.tile([P, KC, HPC, 66], F16)     # col 64 = ones
            nc.vector.memset(v_sb[:, :, :, 64:65], 1.0)
            ctxT = pAB.tile([P, FT, S], F16)
            ow_sb = pAB.tile([P, FT, D], F16)
            nc.gpsimd.dma_start(ow_sb[:], ow_r)

            # phase-A working set (resident until the projection weave ends)
            h_sb = pA.tile([P, DC, S], F16)
            for dc in range(DC):
                nc.sync.dma_start(h_sb[:, dc], hT_r[:, dc])
            cos_sb = pA.tile([P, S], F16)
            nc.scalar.dma_start(cos_sb[:], cosr.ap())
            sin_sb = pA.tile([P, S], F16)
            nc.scalar.dma_start(sin_sb[:], sinr.ap())
            wv_sb = pA.tile([P, DC, FPC], F16)
            nc.gpsimd.dma_start(wv_sb[:], wv_r)

            def emit_qk_chunk(psum_pool, sb_pool, wa, j, t4, qpb_scalar=False):
                """Project + rope one [128-feature, 512-t] chunk of Q or K."""
                tsl = slice(t4 * NQ, (t4 + 1) * NQ)
                pa = psum_pool.tile([P, NQ], F32, tag="paW", name="paW")
                for dc in range(DC):
                    nc.tensor.matmul(pa[:], wa[:, dc], h_sb[:, dc, tsl],
                                     start=(dc == 0), stop=(dc == DC - 1))
                qpb = sb_pool.tile([P, NQ], F16, tag="qpbW")
                if qpb_scalar:
                    nc.scalar.activation(qpb[:], pa[:], AF.Identity,
                                         bias=b4_sb[:, j:j + 1])
                else:
                    nc.vector.tensor_scalar_add(qpb[:], pa[:], b4_sb[:, j:j + 1])
                pr = psum_pool.tile([P, NQ], F32, tag="prW", name="prW")
                nc.tensor.matmul(pr[:], permM_sb[:], qpb[:],
                                 start=True, stop=True)
                t1 = sb_pool.tile([P, NQ], F16, tag="t1W")
                nc.vector.tensor_mul(t1[:], qpb[:], cos_sb[:, tsl])
                t2 = sb_pool.tile([P, NQ], F16, tag="t2W")
                nc.vector.tensor_mul(t2[:], pr[:], sin_sb[:, tsl])
                nc.vector.tensor_add(qk_sb[:, j, tsl], t1[:], t2[:])

            def load_w_tile(sb_pool, j, tag="wW"):
                wa = sb_pool.tile([P, DC, P], F16, tag=tag)
                nc.scalar.dma_start(wa[:], w4_r[:, :, j * P:(j + 1) * P])
                return wa

            # ---------------- Preamble: V (all) + K ft0 + Q ft0 ------------
            with (
                tc.tile_pool(name="pAw", bufs=2) as pAw,
                tc.tile_pool(name="psP", bufs=1, space="PSUM") as psP,
            ):
                wK = load_w_tile(pAw, FT + 0, tag="wK")
                wQ = load_w_tile(pAw, 0, tag="wQ")
                kq = ([(wK, FT + 0, t4) for t4 in range(NQQ)]
                      + [(wQ, 0, t4) for t4 in range(NQQ)])
                for tt in range(KC):
                    pv = psP.tile([P, FPC], F32, tag="pv", name="pv", bufs=2)
                    for dc in range(DC):
                        nc.tensor.matmul(pv[:], h_sb[:, dc, tt * P:(tt + 1) * P],
                                         wv_sb[:, dc], start=(dc == 0),
                                         stop=(dc == DC - 1))
                    nc.scalar.copy(v_sb[:, tt, :, 0:64], pv[:])
                    if tt % 2 == 1:
                        w, j, t4 = kq[tt // 2]
                        emit_qk_chunk(psP, pAw, w, j, t4)

            # ---------------- Phase B with projection/o_proj weave ---------
            # Remaining A work: (K ft, Q ft) for hp 1..3, woven one chunk per
            # kc-block during the previous hp's stream. o_proj for quarter qq
            # woven into quarter qq+1. PSUM: sc 2x2 + ct 2 = 6 banks (B) +
            # 2 banks (weave pa/pr, later o_proj po).
            with (
                tc.tile_pool(name="peb", bufs=1) as peb,
                tc.tile_pool(name="pB", bufs=2) as pB,
                tc.tile_pool(name="psB", bufs=1, space="PSUM") as psB,
                tc.tile_pool(name="pW", bufs=2) as pW,
            ):
                # A-weave chunk list: (j, t4) in the order hp1-needs, hp2, hp3
                aw = []
                for hp in range(1, FT):
                    for j in (FT + hp, hp):
                        aw.append((j, None))           # weight-load marker
                        for t4 in range(NQQ):
                            aw.append((j, t4))
                aw_weights = {}
                awi = 0

                def weave_a():
                    nonlocal awi
                    # emit up to 2 items per call (weight loads are free)
                    budget = 1
                    while budget > 0 and awi < len(aw):
                        j, t4 = aw[awi]
                        if t4 is None:
                            aw_weights[j] = load_w_tile(pW, j, tag=f"wW{j % 2}")
                        else:
                            emit_qk_chunk(psW, pW, aw_weights[j], j, t4)
                            budget -= 1
                        awi += 1

                co = []                                 # pending o_proj chunks
                def weave_c():
                    if co:
                        co.pop(0)()

                def emit_oproj(qq):
                    qsl = slice(qq * NQ, (qq + 1) * NQ)
                    for ot in range(D // P):
                        def emit(ot=ot, qsl=qsl):
                            ptag = "prW" if ot % 2 == 0 else "paW"
                            po = psW.tile([P, NQ], F32, tag=ptag, name="po")
                            for fc in range(FT):
                                nc.tensor.matmul(
                                    po[:], ow_sb[:, fc, ot * P:(ot + 1) * P],
                                    ctxT[:, fc, qsl],
                                    start=(fc == 0), stop=(fc == FT - 1))
                            o_sb = pW.tile([P, NQ], F16, tag="oT", bufs=3)
                            nc.vector.tensor_copy(o_sb[:], po[:])
                            nc.sync.dma_start(
                                outT.ap()[ot * P:(ot + 1) * P, qsl], o_sb[:])
                        co.append(emit)

                with tc.tile_pool(name="psW", bufs=1, space="PSUM") as psW:
                    for hp in range(FT):
                        ft = hp
                        for qq in range(NQQ):
                            qsl = slice(qq * NQ, (qq + 1) * NQ)
                            eb = peb.tile([P, KC, NQ], F16, tag="eb", bufs=2)
                            nc.sync.dma_start(eb[:], eb_r[:, :, qsl])
                            cts = []
                            for hi in range(2):
                                ct = psB.tile([65, NQ], F32, tag=f"ct{hi}",
                                              name=f"ct{hi}")
                                cts.append(ct)
                            prev = None
                            for kc in range(KC):
                                sc = psB.tile([P, 2, NQ], F32, tag="sc",
                                              bufs=2, name="sc")
                                for hi in range(2):
                                    base = 64 * hi
                                    ksl = qk_sb[base:base + 64, FT + ft,
                                                kc * P:(kc + 1) * P]
                                    qop = qk_sb[base:base + 64, ft, qsl]
                                    nc.tensor.matmul(sc[:, hi, :], ksl, qop,
                                                     start=True, stop=True)
                                u = pB.tile([P, 2, NQ], F16, tag="u", bufs=3)
                                nc.scalar.activation(u[:], sc[:], AF.Exp,
                                                     bias=eshift[:])
                                u2 = pB.tile([P, 2, NQ], F16, tag="u2", bufs=3)
                                _, ebb = bass.broadcast_tensor_aps(
                                    u[:], eb[:, kc:kc + 1, :])
                                nc.vector.tensor_mul(u2[:], u[:], ebb)
                                if prev is not None:
                                    for hi in range(2):
                                        h = 2 * hp + hi
                                        nc.tensor.matmul(
                                            cts[hi][:],
                                            v_sb[:, kc - 1, h, 0:65],
                                            prev[:, hi, :], start=(kc == 1),
                                            stop=False)
                                prev = u2
                                # weave: one deferred chunk every 8th block;
                                # once projections are done, slots drain o_proj
                                if kc % 8 == 1:
                                    if awi < len(aw):
                                        weave_a()
                                    else:
                                        weave_c()
                                elif kc % 8 == 5:
                                    weave_c()
                                elif kc % 8 == 3 and awi >= len(aw):
                                    weave_c()
                                elif kc % 8 == 7 and awi >= len(aw):
                                    weave_c()
                            for hi in range(2):
                                h = 2 * hp + hi
                                nc.tensor.matmul(cts[hi][:],
                                                 v_sb[:, KC - 1, h, 0:65],
                                                 prev[:, hi, :], start=False,
                                                 stop=True)
                            # finalize
                            cus = []
                            for hi in range(2):
                                cu = pB.tile([65, NQ], F32, tag=f"cu{hi}")
                                nc.vector.tensor_copy(cu[:], cts[hi][:])
                                cus.append(cu)
                            rscrs, rrecs, rscr2s, rbs = [], [], [], []
                            for hi in range(2):
                                rscr = dpool.tile([NQ], F32)
                                nc.gpsimd.dma_start(rscr[None, :],
                                                    cus[hi][64:65, :])
                                rscrs.append(rscr)
                            rsqs = []
                            for hi in range(2):
                                rsq = pB.tile([32, NQ // 32], F32,
                                              tag=f"rsq{hi}")
                                nc.gpsimd.dma_start(
                                    rsq[:],
                                    rscrs[hi].rearrange("(a b) -> a b", a=32))
                                rsqs.append(rsq)
                            for hi in range(2):
                                rrec = pB.tile([32, NQ // 32], F32,
                                               tag=f"rrec{hi}")
                                nc.vector.reciprocal(rrec[:], rsqs[hi][:])
                                rrecs.append(rrec)
                            for hi in range(2):
                                rscr2 = dpool.tile([NQ], F32)
                                nc.gpsimd.dma_start(
                                    rscr2.rearrange("(a b) -> a b", a=32),
                                    rrecs[hi][:])
                                rscr2s.append(rscr2)
                            for hi in range(2):
                                rb = pB.tile([64, NQ], F32, tag=f"rb{hi}")
                                nc.gpsimd.dma_start(
                                    rb[:], rscr2s[hi].partition_broadcast(64))
                                rbs.append(rb)
                            for hi in range(2):
                                base = 64 * hi
                                nc.vector.tensor_mul(
                                    ctxT[base:base + 64, ft, qsl],
                                    cus[hi][0:64, :], rbs[hi][:])
                            # o_proj for quarter qq becomes ready once the
                            # LAST head-pair (hp==FT-1) finishes it
                            if hp == FT - 1:
                                emit_oproj(qq)
                    # drain remaining weave work
                    while awi < len(aw):
                        weave_a()
                    while co:
                        weave_c()

    nc.compile()
    return nc


def make_core_inputs(hidden_states, attention_bias, rope_cos, rope_sin, head_mask,
                     qkv_w, qkv_b, o_w, S=2048, D=1024):
    """Host-side sharding + layout preparation. Returns list of 8 input dicts."""
    f32 = np.float32
    f16 = np.float16
    hidden_states = np.asarray(hidden_states, f32)
    attention_bias = np.asarray(attention_bias, f32)
    rope_cos = np.asarray(rope_cos, f32)
    rope_sin = np.asarray(rope_sin, f32)
    head_mask = np.asarray(head_mask, f32).reshape(-1)
    qkv_w = np.asarray(qkv_w, f32)
    qkv_b = np.asarray(qkv_b, f32)
    o_w = np.asarray(o_w, f32)

    B = hidden_states.shape[0]
    FPC = HPC * 64
    F = H * 64  # qkv feature dim (row-section size of qkv_w)

    cos_t = rope_cos[0, :, 0, :].T.astype(f32)     # [64, S]
    sin_t = rope_sin[0, :, 0, :].T.astype(f32)
    cosr = np.concatenate([cos_t, cos_t], axis=0)  # [128, S]
    sinr = np.concatenate([sin_t, sin_t], axis=0)

    # rotate_half as a signed permutation: out[c] = -in[c+32] (c%64<32),
    # +in[c-32] (c%64>=32); per 64-row head block, two blocks per 128.
    permM = np.zeros((128, 128), f32)
    for blk in (0, 64):
        for c in range(32):
            permM[blk + c + 32, blk + c] = -1.0
        for c in range(32, 64):
            permM[blk + c - 32, blk + c] = 1.0

    in_maps = []
    ob_extra = {}
    for c in range(8):
        b, g = divmod(c, G)
        fs = slice(g * FPC, (g + 1) * FPC)
        wq = qkv_w[F * 0:F * 1][fs]
        wk = qkv_w[F * 1:F * 2][fs]
        wv = qkv_w[F * 2:F * 3][fs].copy()
        bq = qkv_b[F * 0:F * 1][fs]
        bk = qkv_b[F * 1:F * 2][fs]
        bvv = qkv_b[F * 2:F * 3][fs].copy()
        mask = head_mask[g * HPC:(g + 1) * HPC]
        wv *= np.repeat(mask, 64)[:, None]
        bvv *= np.repeat(mask, 64)
        w4 = np.concatenate([wq.T, wk.T], axis=1)      # [D, 2*FPC]
        b4 = np.concatenate([bq, bk])
        bT = np.ascontiguousarray(attention_bias[b, 0].T)
        ob_extra[c] = o_w[:, g * FPC:(g + 1) * FPC] @ bvv
        m = {
            "hT": np.ascontiguousarray(hidden_states[b].T).astype(f16),
            "w4": np.ascontiguousarray(w4).astype(f16),
            "b4": np.ascontiguousarray(b4),
            "wvT": np.ascontiguousarray(wv.T).astype(f16),
            "cosr": np.ascontiguousarray(cosr).astype(f16),
            "sinr": np.ascontiguousarray(sinr).astype(f16),
            "permM": np.ascontiguousarray(permM).astype(f16),
            "expbT": np.exp(bT).astype(f16),
            "owT": np.ascontiguousarray(o_w[:, g * FPC:(g + 1) * FPC].T).astype(f16),
        }
        in_maps.append(m)
    return in_maps, ob_extra


def kernel(hidden_states, attention_bias, rope_cos, rope_sin, head_mask,
           qkv_w, qkv_b, o_w, o_b, **_unused):
    from concourse.bass_utils import run_bass_kernel_spmd

    B, S, D = hidden_states.shape
    if "nc" not in _CACHE:
        _CACHE["nc"] = build_nc(S=S, D=D)
    nc = _CACHE["nc"]

    in_maps, ob_extra = make_core_inputs(hidden_states, attention_bias, rope_cos,
                                         rope_sin, head_mask, qkv_w, qkv_b, o_w,
                                         S=S, D=D)
    res = run_bass_kernel_spmd(nc, in_maps, list(range(8)))
    _CACHE["last_results"] = res

    o_b = np.asarray(o_b, np.float32)
    out = np.empty((B, S, D), np.float32)
    for b in range(B):
        acc = (res.results[2 * b]["outT"].astype(np.float32).T
               + res.results[2 * b + 1]["outT"].astype(np.float32).T)
        out[b] = acc + (o_b + ob_extra[2 * b] + ob_extra[2 * b + 1])[None, :]
    return out
